# revision 1
# baseline (speedup 1.0000x reference)
"""Trainium2 Bass kernel for nn_GameboyNet — fp8/bf16 mixed, interleaved pipeline.

Sharding: data-parallel over batch, B=8 rows -> 8 NeuronCores.

Numerics (chosen by 32-layer error bisection):
- f32 residual carrier hT (bf16 carrier alone costs 1e-2 rel err at L=32).
- fp8e4 DoubleRow (2x PE): q/k projections, scores (q8,k8), W1 (h8), W2 (u8).
  These are error-neutral even at 32 layers.
- bf16: attention AV and denominators (fp8 V values compound to 2.7e-2 — the
  values are correlated across tokens so quantization error doesn't average).
- f32r: v projection and final conv read the f32 carrier directly at full
  PE speed (N>=256), avoiding a bf16 shadow of h.
- sigmoid -> 0.5*tanh(z/2)+0.5 folded into W2/b2 (host): one ACT table set.
- v-bias folded into b1/b2 (host): no bias matmuls.

Schedule: MLP token-pair chunks and next layer's QKV are interleaved into
the attention window loop (PE executes in emission order; interleaving
keeps it fed while ACT runs exp/tanh). Softmax normalize is pipelined one
window behind; 1/sum via reciprocal_approx_fast + a 1-row PE broadcast
matmul into the spare PSUM bank.
"""
import os
import sys
import types

sys.path.insert(0, '/opt/trn_rl_repo')

import numpy as np
import ml_dtypes

import concourse.bass as bass
import concourse.mybir as mybir
import concourse.tile as tile
from concourse import bacc
from concourse.bass import ds
from concourse.bass_utils import run_bass_kernel_spmd

B, S, D, W, L = 8, 4096, 256, 512, 32
E = 4 * D
NW = S // W
P = 128
DC = D // P
EC = E // P
TP = 4               # token tile-pairs of 1024
TB = S // P          # 32 token blocks of 128
BN_EPS = 1e-5

SQ = 32.0
SK = 32.0
FP8MAX = 224.0

f32 = mybir.dt.float32
f32r = mybir.dt.float32r
bf16 = mybir.dt.bfloat16
fp8 = mybir.dt.float8e4
AF = mybir.ActivationFunctionType
ALU = mybir.AluOpType
DR = mybir.MatmulPerfMode.DoubleRow

LAST_EXEC_NS = None
LAST_TRACE = None

_cache = {}


def _install_ntff_hook():
    try:
        import antenv
        if 'antenv.axon_hooks' in sys.modules:
            return
        mod = types.ModuleType("antenv.axon_hooks")
        _HOOK = [None]
        mod.set_axon_ntff_profile_hook = lambda h: _HOOK.__setitem__(0, h)
        mod.get_axon_ntff_profile_hook = lambda: _HOOK[0]
        sys.modules["antenv.axon_hooks"] = mod
        antenv.axon_hooks = mod
        from trn_agent_boot.trn_boot import _ntff_profile_via_ctypes
        hook = _ntff_profile_via_ctypes('/opt/axon/libaxon_pjrt.so')
        mod.set_axon_ntff_profile_hook(hook)
    except Exception:
        pass


class Net:
    def __init__(self, nc, pools, tiles, scales, n_layers):
        self.nc = nc
        (self.wpool, self.p2, self.accp, self.rbps, self.ps1,
         self.expp, self.rbp, self.tmpp, self.outp) = pools
        (self.hT, self.h8, self.q8, self.k8, self.v_bf, self.u8,
         self.ones_col, self.ebias, self.m45, self.ones_row,
         self.wf_sb, self.bf_sb, self.out_d) = tiles
        self.scales = scales
        self.n_layers = n_layers
        self.wt = {}
        self.flat = lambda ap: ap.rearrange("p a b -> p (a b)")

    # ---- weights ------------------------------------------------------
    def load_weights(self, l):
        nc, dma = self.nc, self.nc.sync.dma_start
        t = dict(
            wq=self.wpool.tile([P, DC, D], fp8, tag="wq", name="wq"),
            wk=self.wpool.tile([P, DC, D], fp8, tag="wk", name="wk"),
            wv=self.wpool.tile([P, DC, D], f32r, tag="wv", name="wv"),
            w1=self.wpool.tile([P, DC, E], fp8, tag="w1", name="w1"),
            w2=self.wpool.tile([P, EC, D], fp8, tag="w2", name="w2"),
            cons=self.wpool.tile([P, 16], f32, tag="cons", name="cons"),
        )
        for kc in range(DC):
            dma(out=t['wq'][:, kc, :], in_=nc.t_wq[ds(l * D + kc * P, P), :])
            dma(out=t['wk'][:, kc, :], in_=nc.t_wk[ds(l * D + kc * P, P), :])
            dma(out=t['wv'][:, kc, :], in_=nc.t_wv[ds(l * D + kc * P, P), :])
            dma(out=t['w1'][:, kc, :], in_=nc.t_w1[ds(l * D + kc * P, P), :])
        for ec in range(EC):
            dma(out=t['w2'][:, ec, :], in_=nc.t_w2[ds(l * E + ec * P, P), :])
        dma(out=t['cons'], in_=nc.t_cons[ds(l * P, P), :])
        self.wt[l] = t
        return t

    # ---- QKV for one token pair (1024 tokens) ------------------------
    def qkv_tp(self, l, tp):
        nc, flat = self.nc, self.flat
        t = self.wt[l]
        al = self.scales[l]
        cons = t['cons']
        t0 = tp * 1024
        for oc in range(DC):
            pq = self.p2.tile([P, 2, 512], f32, tag="t2")
            for i in range(2):
                nc.tensor.matmul(pq[:, i, :], t['wq'][:, :, oc * P:(oc + 1) * P],
                                 self.h8[:, :, t0 + i * 512:t0 + (i + 1) * 512],
                                 start=True, stop=True, perf_mode=DR,
                                 skip_group_check=True)
            nc.scalar.activation(self.q8[:, oc, t0:t0 + 1024], flat(pq[:]),
                                 AF.Identity, bias=cons[:, oc:oc + 1],
                                 scale=al['aq'])
            pk = self.p2.tile([P, 2, 512], f32, tag="t2")
            for i in range(2):
                nc.tensor.matmul(pk[:, i, :], t['wk'][:, :, oc * P:(oc + 1) * P],
                                 self.h8[:, :, t0 + i * 512:t0 + (i + 1) * 512],
                                 start=True, stop=True, perf_mode=DR,
                                 skip_group_check=True)
            nc.vector.tensor_scalar(self.k8[:, oc, t0:t0 + 1024], flat(pk[:]),
                                    al['ak'], cons[:, 2 + oc:3 + oc],
                                    op0=ALU.mult, op1=ALU.add)
        # v: f32r matmul from the f32 carrier, 2 token blocks per 1-bank tile
        for tq in range(4):
            pv = self.ps1.tile([P, 512], f32, tag="v1", name="pv")
            for i in range(2):
                tb = tp * 8 + tq * 2 + i
                for kc in range(DC):
                    nc.tensor.matmul(
                        pv[:, i * 256:i * 256 + 256],
                        self.hT[:, kc, tb * P:(tb + 1) * P],
                        t['wv'][:, kc, :],
                        start=(kc == 0), stop=(kc == 1),
                        skip_group_check=True)
            nc.vector.tensor_copy(
                out=flat(self.v_bf[:, tp * 8 + tq * 2:tp * 8 + tq * 2 + 2, :]),
                in_=pv[:])

    # ---- one attention window (split: scores / AV) -------------------
    def attn_scores(self, l, w):
        nc = self.nc
        al = self.scales[l]
        q0 = w * W
        kstart = (w - 1) * W
        PAIRS = [(0, 0), (2, 0), (4, 0), (6, 256)]
        pairs = PAIRS if w > 0 else PAIRS[2:]
        expT = self.expp.tile([P, 8, 512], bf16, tag="exp", name="expT")
        for (kba, qp) in pairs:
            sp = self.p2.tile([P, 2, 512], f32, tag="t2", name="sp")
            for i, kb in enumerate((kba, kba + 1)):
                kpos = kstart + kb * P
                nc.tensor.matmul(sp[:, i, qp:512],
                                 self.k8[:, :, kpos:kpos + P],
                                 self.q8[:, :, q0 + qp:q0 + W],
                                 start=True, stop=True, perf_mode=DR,
                                 skip_group_check=True)
            nc.scalar.activation(expT[:, kba:kba + 2, qp:512], sp[:, :, qp:512],
                                 AF.Exp, bias=self.ebias[:], scale=al['ax'])
            if kba >= 4:
                nc.vector.tensor_tensor(
                    out=expT[:, kba:kba + 2, qp:qp + 256],
                    in0=expT[:, kba:kba + 2, qp:qp + 256], in1=self.m45[:],
                    op=ALU.mult)
        return (expT, pairs, kstart, q0)

    def attn_av(self, l, w, st):
        nc, flat = self.nc, self.flat
        (expT, pairs, kstart, q0) = st
        nkb = 2 * len(pairs)
        acc = self.accp.tile([P, 2, 512], f32, tag="acc", name="acc")
        ki = 0
        for (kba, qp) in pairs:
            for kb in (kba, kba + 1):
                first, last = (ki == 0), (ki == nkb - 1)
                tb = (kstart + kb * P) // P
                for dc in range(DC):
                    nc.tensor.matmul(acc[:, dc, qp:512],
                                     self.v_bf[:, tb, dc * P:(dc + 1) * P],
                                     expT[:, kb, qp:512],
                                     start=first, stop=last,
                                     skip_group_check=True)
                ki += 1
        sr = self.rbps.tile([P, 512], f32, tag="sr", name="sr")
        ki = 0
        for (kba, qp) in pairs:
            for kb in (kba, kba + 1):
                first, last = (ki == 0), (ki == nkb - 1)
                nc.tensor.matmul(sr[0:1, qp:512], self.ones_col[:, 0:1],
                                 expT[:, kb, qp:512],
                                 start=first, stop=last,
                                 skip_group_check=True)
                ki += 1
        acc_sb = self.tmpp.tile([P, 2, 512], f32, tag="accsb", name="acc_sb")
        nc.vector.tensor_copy(out=flat(acc_sb[:]), in_=flat(acc[:]))
        recip = self.rbp.tile([1, 512], f32, tag="recip", name="recip")
        nc.vector.reciprocal_approx_fast(out=recip[0:1, :], in_=sr[0:1, :])
        recip_bf = self.rbp.tile([1, 512], bf16, tag="recipbf", name="recip_bf")
        nc.vector.tensor_copy(out=recip_bf[0:1, :], in_=recip[0:1, :])
        rb = self.rbps.tile([P, 512], f32, tag="sr", name="rb")
        nc.tensor.matmul(rb[:], self.ones_row[0:1, :], recip_bf[0:1, :],
                         start=True, stop=True, skip_group_check=True)
        return (acc_sb, rb, q0)

    def emit_norm(self, acc_sb, rb, q0):
        nc = self.nc
        o_f = self.tmpp.tile([P, 2, 512], f32, tag="obf")
        for dc in range(DC):
            nc.vector.tensor_tensor(out=o_f[:, dc, :], in0=acc_sb[:, dc, :],
                                    in1=rb[:], op=ALU.mult)
        nc.vector.tensor_tensor(out=self.hT[:, :, q0:q0 + W],
                                in0=self.hT[:, :, q0:q0 + W],
                                in1=o_f[:], op=ALU.add)
        nc.vector.tensor_copy(out=self.h8[:, :, q0:q0 + W],
                              in_=self.hT[:, :, q0:q0 + W])

    # ---- MLP chunks for one token pair -------------------------------
    def mlp_w1(self, l, tp, ecs):
        nc, flat = self.nc, self.flat
        t, al = self.wt[l], self.scales[l]
        t0 = tp * 1024
        for ec in ecs:
            pu = self.p2.tile([P, 2, 512], f32, tag="t2")
            for i in range(2):
                nc.tensor.matmul(pu[:, i, :], t['w1'][:, :, ec * P:(ec + 1) * P],
                                 self.h8[:, :, t0 + i * 512:t0 + (i + 1) * 512],
                                 start=True, stop=True, perf_mode=DR,
                                 skip_group_check=True)
            nc.scalar.activation(self.u8[:, ec, t0:t0 + 1024], flat(pu[:]),
                                 AF.Tanh, bias=t['cons'][:, 4 + ec:5 + ec],
                                 scale=al['a1'])

    def mlp_w2(self, l, tp, dc):
        nc, flat = self.nc, self.flat
        t, al = self.wt[l], self.scales[l]
        cons = t['cons']
        t0 = tp * 1024
        pm = self.p2.tile([P, 2, 512], f32, tag="t2")
        for i in range(2):
            for ee in range(4):
                nc.tensor.matmul(pm[:, i, :],
                                 t['w2'][:, 2 * ee:2 * ee + 2, dc * P:(dc + 1) * P],
                                 self.u8[:, 2 * ee:2 * ee + 2,
                                         t0 + i * 512:t0 + (i + 1) * 512],
                                 start=(ee == 0), stop=(ee == 3), perf_mode=DR,
                                 skip_group_check=True)
        s1 = self.tmpp.tile([P, 1024], f32, tag="s1")
        nc.vector.scalar_tensor_tensor(out=s1[:], in0=flat(pm[:]), scalar=al['a2'],
                                       in1=self.hT[:, dc, t0:t0 + 1024],
                                       op0=ALU.mult, op1=ALU.add)
        nc.scalar.activation(self.hT[:, dc, t0:t0 + 1024], s1[:], AF.Identity,
                             bias=cons[:, 14 + dc:15 + dc],
                             scale=cons[:, 12 + dc:13 + dc])
        nc.vector.tensor_copy(out=self.h8[:, dc, t0:t0 + 1024],
                              in_=self.hT[:, dc, t0:t0 + 1024])

    def final_tp(self, tp):
        nc = self.nc
        t0 = tp * 1024
        for oc in range(DC):
            pf = self.p2.tile([P, 2, 512], f32, tag="t2")
            for i in range(2):
                for kc in range(DC):
                    nc.tensor.matmul(pf[:, i, :],
                                     self.wf_sb[:, kc, oc * P:(oc + 1) * P],
                                     self.hT[:, kc, t0 + i * 512:t0 + (i + 1) * 512],
                                     start=(kc == 0), stop=(kc == 1),
                                     skip_group_check=True)
            ot = self.outp.tile([P, 1024], f32, tag="out")
            nc.scalar.activation(ot[:], pf[:].rearrange("p a b -> p (a b)"),
                                 AF.Relu, bias=self.bf_sb[:, oc:oc + 1], scale=1.0)
            nc.sync.dma_start(out=self.out_d[oc * P:(oc + 1) * P, t0:t0 + 1024],
                              in_=ot[:])

    # ---- full network -------------------------------------------------
    def emit(self):
        nl = self.n_layers
        self.load_weights(0)
        if nl > 1:
            self.load_weights(1)
        for tp in range(TP):
            self.qkv_tp(0, tp)
        queue = []   # (avail_global_window, fn) — persists across layers
        for l in range(nl):
            if l + 2 < nl:
                self.load_weights(l + 2)
            last = (l + 1 == nl)

            for tp in range(TP):
                av = l * NW + 2 * tp + 2
                queue.append((av, lambda l=l, tp=tp: self.mlp_w1(l, tp, range(0, 4))))
                queue.append((av, lambda l=l, tp=tp: self.mlp_w1(l, tp, range(4, 8))))
                queue.append((av, lambda l=l, tp=tp: self.mlp_w2(l, tp, 0)))
                queue.append((av, lambda l=l, tp=tp: self.mlp_w2(l, tp, 1)))
                if last:
                    queue.append((av + 2, lambda tp=tp: self.final_tp(tp)))
                else:
                    queue.append((av + 2, lambda l=l, tp=tp: self.qkv_tp(l + 1, tp)))

            queue.sort(key=lambda e: e[0])
            pend = None
            for w in range(NW):
                gw = l * NW + w
                st = self.attn_scores(l, w)
                if pend is not None:
                    self.emit_norm(*pend)
                    pend = None
                budget = 3
                while queue and queue[0][0] <= gw and budget > 0:
                    queue.pop(0)[1]()
                    budget -= 1
                pend = self.attn_av(l, w, st)
            self.emit_norm(*pend)
            if l - 3 in self.wt:
                del self.wt[l - 3]
        while queue:
            queue.pop(0)[1]()


def _build(scales, n_layers=L):
    nc = bacc.Bacc("TRN2", target_bir_lowering=False, debug=False)

    nc.t_h0 = nc.dram_tensor("h0T", [D, S], f32r, kind="ExternalInput")
    nc.t_h08 = nc.dram_tensor("h08T", [D, S], fp8, kind="ExternalInput")
    nc.t_wq = nc.dram_tensor("wq8", [n_layers * D, D], fp8, kind="ExternalInput")
    nc.t_wk = nc.dram_tensor("wk8", [n_layers * D, D], fp8, kind="ExternalInput")
    nc.t_wv = nc.dram_tensor("wvf", [n_layers * D, D], f32r, kind="ExternalInput")
    nc.t_w1 = nc.dram_tensor("w18", [n_layers * D, E], fp8, kind="ExternalInput")
    nc.t_w2 = nc.dram_tensor("w28", [n_layers * E, D], fp8, kind="ExternalInput")
    nc.t_cons = nc.dram_tensor("cons", [n_layers * P, 16], f32, kind="ExternalInput")
    m45_d = nc.dram_tensor("m45", [P, 512], bf16, kind="ExternalInput")
    wf_d = nc.dram_tensor("wff", [D, D], f32r, kind="ExternalInput")
    bf_d = nc.dram_tensor("bfc", [P, DC], f32, kind="ExternalInput")
    out_d = nc.dram_tensor("outT", [D, S], f32, kind="ExternalOutput")

    with tile.TileContext(nc) as tc:
        with tc.tile_pool(name="persist", bufs=1) as persist, \
             tc.tile_pool(name="wpool", bufs=4) as wpool, \
             tc.tile_pool(name="p2", bufs=2, space="PSUM") as p2, \
             tc.tile_pool(name="accp", bufs=1, space="PSUM") as accp, \
             tc.tile_pool(name="rbps", bufs=1, space="PSUM") as rbps, \
             tc.tile_pool(name="ps1", bufs=1, space="PSUM") as ps1, \
             tc.tile_pool(name="expp", bufs=2) as expp, \
             tc.tile_pool(name="rbp", bufs=2) as rbp, \
             tc.tile_pool(name="tmpp", bufs=2) as tmpp, \
             tc.tile_pool(name="outp", bufs=2) as outp:

            hT = persist.tile([P, DC, S], f32r)
            h8 = persist.tile([P, DC, S], fp8)
            q8 = persist.tile([P, DC, S], fp8)
            k8 = persist.tile([P, DC, S], fp8)
            v_bf = persist.tile([P, TB, D], bf16)
            u8 = persist.tile([P, EC, S], fp8)
            ones_col = persist.tile([P, 1], bf16)
            ebias = persist.tile([P, 1], f32)
            m45 = persist.tile([P, 2, 256], bf16)
            ones_row = persist.tile([1, P], bf16)
            wf_sb = persist.tile([P, DC, D], f32r)
            bf_sb = persist.tile([P, DC], f32)

            nc.vector.memset(ones_col, 1.0)
            nc.vector.memset(ebias, 0.0)
            nc.vector.memset(ones_row, 1.0)
            nc.sync.dma_start(out=m45[:].rearrange("p a b -> p (a b)"), in_=m45_d[:, :])
            for kc in range(DC):
                nc.sync.dma_start(out=hT[:, kc, :], in_=nc.t_h0[kc * P:(kc + 1) * P, :])
                nc.sync.dma_start(out=h8[:, kc, :], in_=nc.t_h08[kc * P:(kc + 1) * P, :])
                nc.sync.dma_start(out=wf_sb[:, kc, :], in_=wf_d[kc * P:(kc + 1) * P, :])
            nc.sync.dma_start(out=bf_sb, in_=bf_d[:, :])

            pools = (wpool, p2, accp, rbps, ps1, expp, rbp, tmpp, outp)
            tiles = (hT, h8, q8, k8, v_bf, u8, ones_col, ebias, m45, ones_row,
                     wf_sb, bf_sb, out_d)
            Net(nc, pools, tiles, scales, n_layers).emit()

    nc.compile()
    return nc


def _q8(x, s):
    return np.clip(x * s, -240.0, 240.0).astype(ml_dtypes.float8_e4m3fn)


def _prep_host(inputs, n_layers=L):
    x = np.asarray(inputs['x'])
    emb = np.asarray(inputs['emb'], np.float32)
    scale = 1.0 / np.sqrt(D)
    bn_scale = 1.0 / np.sqrt(1.0 + BN_EPS)

    g = lambda k: np.asarray(inputs[k], np.float32)[:n_layers]
    Wq, Wk, Wv, W1, W2 = g('Wq'), g('Wk'), g('Wv'), g('W1'), g('W2')
    bq, bk, bv, b1, b2 = g('bq'), g('bk'), g('bv'), g('b1'), g('b2')
    gamma, beta = g('gamma'), g('beta')

    b1f = b1 + np.einsum('led,ld->le', W1, bv)
    W2e = 0.5 * W2
    b2f = b2 + bv + W2e.sum(axis=2)

    A = gamma * bn_scale
    C = A * b2f + beta

    sw = lambda M: FP8MAX / np.abs(M).max(axis=(1, 2))
    swq, swk, sw1, sw2 = sw(Wq), sw(Wk), sw(W1), sw(W2e)

    wq8 = np.stack([_q8(Wq[l].T, swq[l]) for l in range(n_layers)]) \
        .reshape(n_layers * D, D)
    wk8 = np.stack([_q8(Wk[l].T, swk[l]) for l in range(n_layers)]) \
        .reshape(n_layers * D, D)
    wvf = np.ascontiguousarray(
        np.transpose(Wv, (0, 2, 1)).reshape(n_layers * D, D)).astype(np.float32)
    w18 = np.stack([_q8(W1[l].T, sw1[l]) for l in range(n_layers)]) \
        .reshape(n_layers * D, E)
    w28 = np.stack([_q8(W2e[l].T, sw2[l]) for l in range(n_layers)]) \
        .reshape(n_layers * E, D)

    scales = []
    for l in range(n_layers):
        scales.append(dict(
            aq=float(SQ * scale / swq[l]),
            ak=float(SK / swk[l]),
            ax=float(1.0 / (SQ * SK)),
            a1=float(1.0 / (2.0 * sw1[l])),
            a2=float(1.0 / sw2[l]),
        ))

    cons = np.zeros((n_layers, P, 16), np.float32)
    cons[:, :, 0:2] = (bq * SQ * scale).reshape(n_layers, DC, P).transpose(0, 2, 1)
    cons[:, :, 2:4] = (bk * SK).reshape(n_layers, DC, P).transpose(0, 2, 1)
    cons[:, :, 4:12] = (b1f * 0.5).reshape(n_layers, EC, P).transpose(0, 2, 1)
    cons[:, :, 12:14] = A.reshape(n_layers, DC, P).transpose(0, 2, 1)
    cons[:, :, 14:16] = C.reshape(n_layers, DC, P).transpose(0, 2, 1)
    cons = cons.reshape(n_layers * P, 16)

    r = np.arange(P)
    tri = (r[None, :] >= r[:, None]).astype(np.float32)
    m45 = np.zeros((P, 2, 256), np.float32)
    m45[:, 0, 0:128] = tri
    m45[:, 0, 128:256] = 1.0
    m45[:, 1, 128:256] = tri
    m45 = m45.reshape(P, 512).astype(ml_dtypes.bfloat16)

    Wf = np.asarray(inputs['Wf'], np.float32)
    wff = np.ascontiguousarray(Wf.T).astype(np.float32)
    bfc = np.asarray(inputs['bf'], np.float32).reshape(DC, P).T.copy()

    shared = dict(wq8=wq8, wk8=wk8, wvf=wvf, w18=w18, w28=w28,
                  cons=cons, m45=m45, wff=wff, bfc=bfc)

    h0 = emb[x]
    in_maps = []
    for b in range(B):
        m = dict(shared)
        hb = np.ascontiguousarray(h0[b].T)     # (D, S)
        m['h0T'] = hb.astype(np.float32)
        m['h08T'] = _q8(hb, 1.0)
        in_maps.append(m)
    return in_maps, scales


def kernel(**inputs):
    global LAST_EXEC_NS, LAST_TRACE
    n_layers = int(os.environ.get('KERNEL_NLAYERS', L))
    trace = os.environ.get('KERNEL_TRACE', '0') == '1'
    if trace:
        _install_ntff_hook()

    in_maps, scales = _prep_host(inputs, n_layers=n_layers)
    key = n_layers
    if key not in _cache:
        _cache[key] = _build(scales, n_layers=n_layers)
    nc = _cache[key]

    res = run_bass_kernel_spmd(nc, in_maps, core_ids=list(range(B)), trace=trace)
    LAST_EXEC_NS = res.exec_time_ns
    LAST_TRACE = res.instructions_and_trace[1] if res.instructions_and_trace else None
    out = np.stack([res.results[b]['outT'] for b in range(B)], axis=0)
    return out

